# revision 19
# baseline (speedup 1.0000x reference)
"""Trainium2 Bass kernel for the CIN (xDeepFM) block.

inputs [2048,39,16] f32, W0 [1521,128], W1 [4992,128] -> out [2048,256] f32.
Data-parallel over the batch axis across 8 NeuronCores; weights replicated.

Per-core pipeline (R = 256 batches x 16 dims = 4096 rows r=(b,d)):
  X1[r,o] = sum_{m<n} w0sym[(m,n),o] x_m x_n + sum_m w0x[m,o] x_m^2 via the
  polarization identity: PE builds pair-sums s=(x_m+x_n) for 741 off-diag
  pairs (6 K-tiles of 128), ACT squares them during PSUM evacuation, PE
  contracts with 0.5*w0sym; the diagonal + correction terms use xsq = x*x
  (computed SBUF->SBUF on the Pool engine) as a 7th K=39 accumulation tile.
  Layer 1 contracts d first: per-batch Gram H_b = X1_b^T [x_b | 1] via
  K=16 matmuls on partition-offset slices (compact x tile, no zero pad);
  out2 accumulates H^T-slices against W1; out1 comes from the ones column
  via a PE transpose. All matmul operands bf16, accumulation fp32.

PE program order is software-pipelined (sel(rc+1); gram(rc-1); x1(rc)) so
the ACT square latency never stalls the PE, and the For_i timing loop wraps
a 2-unrolled body so each iteration's DMAs overlap the other's compute.
"""


import ml_dtypes
import numpy as np

BF16 = ml_dtypes.bfloat16

B, M0, D = 2048, 39, 16
C0, C1 = 128, 128
NCORES = 8
BL = B // NCORES          # 256 batches per core
R = BL * D                # 4096 rows per core
NPAIR = (M0 * (M0 - 1)) // 2   # 741 off-diagonal pairs
KT = (NPAIR + 127) // 128      # 6 K-tiles
RC = 512                       # r-chunk for the sel/square stage
NRC = R // RC                  # 8
NCHUNK = R // 128              # 32 chunks of (8 b x 16 d)
BPC = 128 // D                 # 8 batches per 128-row chunk
HW_ = M0 + 1                   # x columns + ones column (d-sum -> out1)


def host_constants(W0, W1):
    """Core-independent prepped tensors.

    Polarization: x_m*x_n = 0.5*(x_m+x_n)^2 - 0.5*(x_m^2+x_n^2), so
      X1[r,:] = sum_p (0.5*w0sym[p])*sq_p[r] + sum_m w0x[m]*x_m[r]^2
    with sq_p built on the PE via the summed selection matrix and squared
    during the ACT PSUM->SBUF evacuation, and w0x carrying both the
    diagonal W0 terms and the -0.5*(x_m^2+x_n^2) corrections.
    """
    pairs = [(m, n) for m in range(M0) for n in range(m + 1, M0)]
    assert len(pairs) == NPAIR

    selsum = np.zeros((128, KT, 128), dtype=np.float32)
    for p, (m, n) in enumerate(pairs):
        t, q = divmod(p, 128)
        selsum[m, t, q] += 1.0
        selsum[n, t, q] += 1.0

    W0r = W0.reshape(M0, M0, C0)
    w0sym = np.zeros((KT * 128, C0), dtype=np.float32)
    for p, (m, n) in enumerate(pairs):
        w0sym[p] = W0r[m, n] + W0r[n, m]
    w0h = 0.5 * w0sym.reshape(KT, 128, C0).transpose(1, 0, 2).copy()

    w0x = np.zeros((128, C0), dtype=np.float32)
    for m in range(M0):
        w0x[m] = W0r[m, m]
    for p, (m, n) in enumerate(pairs):
        w0x[m] -= 0.5 * w0sym[p]
        w0x[n] -= 0.5 * w0sym[p]

    # w1sb[n, m, o] = W1[m*128+n, o]
    w1sb = W1.reshape(M0, C1, C0).transpose(1, 0, 2).copy()

    return dict(
        selsum=np.ascontiguousarray(selsum.astype(BF16)),
        w0h=np.ascontiguousarray(w0h.astype(BF16)),
        w0x=np.ascontiguousarray(w0x.astype(BF16)),
        w1sb=np.ascontiguousarray(w1sb.astype(BF16)),
        ident=np.ascontiguousarray(np.eye(128, dtype=np.float32).astype(BF16)),
    )


def host_core_inputs(x_c):
    """Per-core prepped tensors from the [BL, M0, D] input shard."""
    # xdT[m, ch*128 + b8*16 + d] = x[ch*8+b8, m, d]  (zero-padded to 128
    # partitions so every K-dim is a full 128)
    xdT = np.zeros((128, R), dtype=np.float32)
    xdT[:M0] = x_c.transpose(1, 0, 2).reshape(M0, R)
    xdT = np.ascontiguousarray(xdT.astype(BF16))
    # 8-way block-diagonal Gram operand (single K=128 matmul per chunk):
    # xc[b8*16+d, ch, b8*40+m] = x[ch*8+b8, m, d]; col m=39 holds 1.0
    xt = x_c.reshape(NCHUNK, BPC, M0, D).transpose(1, 3, 0, 2)  # [b8, d, ch, m]
    xc = np.zeros((BPC, D, NCHUNK, BPC, HW_), dtype=np.float32)
    for b8 in range(BPC):
        xc[b8, :, :, b8, :M0] = xt[b8]
        xc[b8, :, :, b8, M0] = 1.0
    xc = xc.reshape(128, NCHUNK, BPC * HW_)
    return dict(
        xdT=xdT,
        xc=np.ascontiguousarray(xc.astype(BF16)),
    )


def split_sync_waits(nc):
    """Rewrite every instruction carrying >1 sync wait: keep the first wait,
    hoist the rest onto same-engine NoOps inserted immediately before it.
    (walrus in this toolchain rejects >1 sync-wait per instruction.)"""
    import concourse.mybir as mybir

    counter = [0]
    for f in nc.m.functions:
        for bb in f.blocks:
            new_list = []
            changed = False
            for inst in bb.instructions:
                si = inst.sync_info
                waits = list(si.on_wait) if si is not None else []
                if len(waits) > 1:
                    changed = True
                    for w in waits[:-1]:
                        counter[0] += 1
                        nop = mybir.InstNoOp(
                            name=f"WSPLIT-{counter[0]}", ins=[], outs=[]
                        )
                        nop.engine = inst.engine
                        nop.sync_info = mybir.SyncInfo(on_wait=[w], on_update=[])
                        new_list.append(nop)
                    si.on_wait = waits[-1:]
                new_list.append(inst)
            if changed:
                bb.instructions = new_list
    return counter[0]


def build_program(reps=1, split_waits=True, loop_reps=None, cfg=None):
    """loop_reps: if set, wrap a 2-unrolled body in a tc.For_i hardware loop
    (loop_reps must be even; total reps executed = loop_reps) for slope-based
    HW timing with cross-iteration DMA/compute overlap."""
    import contextlib

    cfg = cfg or {}

    import concourse.bass as bass
    import concourse.mybir as mybir
    import concourse.tile as tile

    f32 = mybir.dt.float32
    bf16 = mybir.dt.bfloat16

    if loop_reps is not None:
        assert loop_reps % 2 == 0, "loop_reps must be even (2-unrolled body)"
        body_reps, trips = 2, loop_reps // 2
    else:
        body_reps, trips = reps, None

    nc = bass.Bass("TRN2", target_bir_lowering=False, debug=False)
    d_xdT = nc.dram_tensor("xdT", [128, R], bf16, kind="ExternalInput")
    d_xc = nc.dram_tensor("xc", [128, NCHUNK, BPC * HW_], bf16, kind="ExternalInput")
    d_sel = nc.dram_tensor("selsum", [128, KT, 128], bf16, kind="ExternalInput")
    d_w0 = nc.dram_tensor("w0h", [128, KT, C0], bf16, kind="ExternalInput")
    d_w0x = nc.dram_tensor("w0x", [128, C0], bf16, kind="ExternalInput")
    d_w1 = nc.dram_tensor("w1sb", [128, M0, C0], bf16, kind="ExternalInput")
    d_id = nc.dram_tensor("ident", [128, 128], bf16, kind="ExternalInput")
    d_out = nc.dram_tensor("out", [BL, C0 + C1], f32, kind="ExternalOutput")

    with tile.TileContext(nc) as tc:
        with (
            tc.tile_pool(name="const", bufs=2) as cpool,
            tc.tile_pool(name="xsqp", bufs=2) as xsqpool,
            tc.tile_pool(name="sqp", bufs=2) as sqpool,
            tc.tile_pool(name="x1p", bufs=2) as x1pool,
            tc.tile_pool(name="hp", bufs=2) as hpool,
            tc.tile_pool(name="outp", bufs=2) as opool,
            tc.tile_pool(name="psA", bufs=2, space="PSUM") as psA,
            tc.tile_pool(name="psB", bufs=2, space="PSUM") as psB,
            tc.tile_pool(name="psC", bufs=2, space="PSUM") as psC,
        ):
            loop_cm = (
                tc.For_i(
                    0,
                    trips,
                    1,
                    hint_engines=(
                        mybir.EngineType.PE,
                        mybir.EngineType.Activation,
                        mybir.EngineType.DVE,
                        mybir.EngineType.SP,
                        mybir.EngineType.Pool,
                    ),
                )
                if trips is not None
                else contextlib.nullcontext()
            )
            with loop_cm:
                for _rep in range(body_reps):
                    xdT = cpool.tile([128, R], bf16, tag="xdT")
                    sel = cpool.tile([128, KT, 128], bf16, tag="sel")
                    w0 = cpool.tile([128, KT, C0], bf16, tag="w0")
                    w0x = cpool.tile([128, C0], bf16, tag="w0x")
                    w1 = cpool.tile([128, M0, C0], bf16, tag="w1")
                    xc = cpool.tile([128, NCHUNK, BPC * HW_], bf16, tag="xc")
                    ident = cpool.tile([128, 128], bf16, tag="ident")
                    # first compute's operands first
                    nc.sync.dma_start(xdT[:, : R // 2], d_xdT[:, : R // 2])
                    nc.sync.dma_start(sel[:], d_sel[:, :, :])
                    nc.sync.dma_start(w0[:], d_w0[:, :, :])
                    nc.sync.dma_start(xdT[:, R // 2 :], d_xdT[:, R // 2 :])
                    nc.sync.dma_start(
                        xc[:, : NCHUNK // 2, :], d_xc[:, : NCHUNK // 2, :]
                    )
                    nc.sync.dma_start(
                        xc[:, NCHUNK // 2 :, :], d_xc[:, NCHUNK // 2 :, :]
                    )
                    nc.sync.dma_start(w0x[:], d_w0x[:, :])
                    nc.sync.dma_start(w1[:, : M0 // 2, :], d_w1[:, : M0 // 2, :])
                    nc.sync.dma_start(w1[:, M0 // 2 :, :], d_w1[:, M0 // 2 :, :])
                    nc.sync.dma_start(ident[:], d_id[:, :])

                    xsq = xsqpool.tile([128, R], bf16, tag="xsq")
                    x1sb = x1pool.tile([128, NCHUNK, C0], bf16, tag="x1sb")
                    hsb = hpool.tile([128, NCHUNK, BPC * HW_], bf16, tag="hsb")
                    hsb3 = hsb[:].rearrange("n c (b m) -> n (c b) m", m=HW_)
                    outsb = opool.tile([128, 2, C0 + C1], f32, tag="outsb")

                    XSQ_ENGINE = (
                        nc.gpsimd.tensor_mul
                        if cfg.get("xsq_pool", False)
                        else nc.vector.tensor_mul
                    )
                    OUT_DMA_QUEUE = (
                        nc.gpsimd.dma_start
                        if cfg.get("out_pool", False)
                        else nc.sync.dma_start
                    )
                    sqs = {}

                    def emit_sel_dual(rc, g):
                        rsl = slice(rc * RC, (rc + 1) * RC)
                        ps = psA.tile([128, 2, RC], f32, tag="sel2")
                        for j in range(2):
                            nc.tensor.matmul(
                                ps[:, j, :],
                                sel[:, 2 * g + j, :],
                                xdT[:, rsl],
                                start=True,
                                stop=True,
                            )
                        sq = sqpool.tile([128, 2, RC], bf16, tag=f"sq{g}")
                        nc.scalar.square(sq[:], ps[:])
                        sqs[(rc, g)] = sq

                    def emit_xsq(rc):
                        rsl = slice(rc * RC, (rc + 1) * RC)
                        # xsq = x*x (SBUF->SBUF)
                        XSQ_ENGINE(
                            xsq[:, rsl], xdT[:, rsl], xdT[:, rsl]
                        )

                    def emit_x1_chunk(x1ps, rc, rs):
                        ch = rc * (RC // 128) + rs
                        csl = slice(rs * 128, (rs + 1) * 128)
                        for t in range(KT):
                            nc.tensor.matmul(
                                x1ps[:, rs, :],
                                sqs[(rc, t // 2)][:, t % 2, csl],
                                w0[:, t, :],
                                start=(t == 0),
                                stop=False,
                            )
                        nc.tensor.matmul(
                            x1ps[:, rs, :],
                            xsq[:, ch * 128 : (ch + 1) * 128],
                            w0x[:],
                            start=False,
                            stop=True,
                        )

                    def emit_gram_chunk(ch):
                        hps = psC.tile([128, BPC * HW_], f32, tag="h")
                        nc.tensor.matmul(
                            hps[:],
                            x1sb[:, ch, :],
                            xc[:, ch, :],
                            start=True,
                            stop=True,
                        )
                        nc.vector.tensor_copy(hsb[:, ch, :], hps[:])

                    def emit_out(bt):
                        btsl = slice(bt * 128, (bt + 1) * 128)
                        o2 = psB.tile([128, C1], f32, tag="x1")
                        for m in range(M0):
                            nc.tensor.matmul(
                                o2[:],
                                hsb3[:, btsl, m],
                                w1[:, m, :],
                                start=(m == 0),
                                stop=(m == M0 - 1),
                            )
                        nc.vector.tensor_copy(outsb[:, bt, C0:], o2[:])
                        o1 = psA.tile([128, C0], bf16, tag="sel2")
                        nc.tensor.transpose(o1[:], hsb3[:, btsl, M0], ident[:])
                        nc.vector.tensor_copy(outsb[:, bt, :C0], o1[:])
                        # store on a side queue so next-iteration input
                        # loads on the sync queue aren't blocked behind it
                        OUT_DMA_QUEUE(d_out[btsl, :], outsb[:, bt, :])

                    # software-pipelined schedule, period rc (RC=512 rows);
                    # gram chunks of rc-1 and x1 chunks of rc are spread
                    # through the period so no engine gates the PE queue head:
                    #   d0 g0 d1 g1 d2 g2 x1c0 g3 x1c1 x1c2 x1c3
                    nch = RC // 128
                    for g in range(3):
                        emit_sel_dual(0, g)
                    emit_xsq(0)
                    for rc in range(NRC):
                        if rc + 1 < NRC:
                            emit_xsq(rc + 1)
                        x1ps = psB.tile([128, nch, C0], f32, tag="x1")
                        gram_base = (rc - 1) * nch

                        def gram(i):
                            if rc - 1 >= 0:
                                emit_gram_chunk(gram_base + i)

                        for g in range(3):
                            if rc + 1 < NRC:
                                emit_sel_dual(rc + 1, g)
                            gram(g)
                        emit_x1_chunk(x1ps, rc, 0)
                        gram(3)
                        for rs in range(1, nch):
                            emit_x1_chunk(x1ps, rc, rs)
                        nc.vector.tensor_copy(
                            x1sb[:, rc * nch : (rc + 1) * nch, :], x1ps[:]
                        )
                        if rc == NRC // 2:
                            emit_out(0)
                    for i in range(nch):
                        emit_gram_chunk((NRC - 1) * nch + i)
                    emit_out(1)

    if split_waits:
        split_sync_waits(nc)
    return nc


def make_in_maps(inputs, W0, W1):
    consts = host_constants(np.asarray(W0), np.asarray(W1))
    in_maps = []
    for c in range(NCORES):
        x_c = np.ascontiguousarray(np.asarray(inputs)[c * BL : (c + 1) * BL])
        m = dict(consts)
        m.update(host_core_inputs(x_c))
        in_maps.append(m)
    return in_maps


_KERNEL_CACHE = {}


def kernel(inputs, W0, W1):
    inputs = np.ascontiguousarray(np.asarray(inputs, dtype=np.float32))
    W0 = np.ascontiguousarray(np.asarray(W0, dtype=np.float32))
    W1 = np.ascontiguousarray(np.asarray(W1, dtype=np.float32))
    assert inputs.shape == (B, M0, D) and W0.shape == (M0 * M0, C0)
    assert W1.shape == (M0 * C0, C1)

    if "nc" not in _KERNEL_CACHE:
        _KERNEL_CACHE["nc"] = build_program()
    nc = _KERNEL_CACHE["nc"]

    in_maps = make_in_maps(inputs, W0, W1)

    from concourse.bass_utils import run_bass_kernel_spmd

    res = run_bass_kernel_spmd(nc, in_maps, core_ids=list(range(NCORES)))
    out = np.concatenate([res.results[c]["out"] for c in range(NCORES)], axis=0)
    return np.ascontiguousarray(out.astype(np.float32))


# revision 20
# speedup vs baseline: 1.2005x; 1.2005x over previous
"""Trainium2 Bass kernel for the CIN (xDeepFM) block.

inputs [2048,39,16] f32, W0 [1521,128], W1 [4992,128] -> out [2048,256] f32.
Data-parallel over the batch axis across 8 NeuronCores; weights replicated.

Per-core pipeline (R = 256 batches x 16 dims = 4096 rows r=(b,d)):
  X1[r,o] = sum_{m<n} w0sym[(m,n),o] x_m x_n + sum_m w0x[m,o] x_m^2 via the
  polarization identity: PE builds pair-sums s=(x_m+x_n) for 741 off-diag
  pairs (6 K-tiles of 128), ACT squares them during PSUM evacuation, PE
  contracts with 0.5*w0sym; the diagonal + correction terms use xsq = x*x
  (computed SBUF->SBUF on the Pool engine) as a 7th K=39 accumulation tile.
  Layer 1 contracts d first: per-batch Gram H_b = X1_b^T [x_b | 1] via
  K=16 matmuls on partition-offset slices (compact x tile, no zero pad);
  out2 accumulates H^T-slices against W1; out1 comes from the ones column
  via a PE transpose. All matmul operands bf16, accumulation fp32.

PE program order is software-pipelined (sel(rc+1); gram(rc-1); x1(rc)) so
the ACT square latency never stalls the PE, and the For_i timing loop wraps
a 2-unrolled body so each iteration's DMAs overlap the other's compute.
"""


import ml_dtypes
import numpy as np

BF16 = ml_dtypes.bfloat16

B, M0, D = 2048, 39, 16
C0, C1 = 128, 128
NCORES = 8
BL = B // NCORES          # 256 batches per core
R = BL * D                # 4096 rows per core
NPAIR = (M0 * (M0 - 1)) // 2   # 741 off-diagonal pairs
KT = (NPAIR + 127) // 128      # 6 K-tiles
RC = 512                       # r-chunk for the sel/square stage
NRC = R // RC                  # 8
NCHUNK = R // 128              # 32 chunks of (8 b x 16 d)
BPC = 128 // D                 # 8 batches per 128-row chunk
HW_ = M0 + 1                   # x columns + ones column (d-sum -> out1)


def host_constants(W0, W1):
    """Core-independent prepped tensors.

    Polarization: x_m*x_n = 0.5*(x_m+x_n)^2 - 0.5*(x_m^2+x_n^2), so
      X1[r,:] = sum_p (0.5*w0sym[p])*sq_p[r] + sum_m w0x[m]*x_m[r]^2
    with sq_p built on the PE via the summed selection matrix and squared
    during the ACT PSUM->SBUF evacuation, and w0x carrying both the
    diagonal W0 terms and the -0.5*(x_m^2+x_n^2) corrections.
    """
    pairs = [(m, n) for m in range(M0) for n in range(m + 1, M0)]
    assert len(pairs) == NPAIR

    selsum = np.zeros((128, KT, 128), dtype=np.float32)
    for p, (m, n) in enumerate(pairs):
        t, q = divmod(p, 128)
        selsum[m, t, q] += 1.0
        selsum[n, t, q] += 1.0

    W0r = W0.reshape(M0, M0, C0)
    w0sym = np.zeros((KT * 128, C0), dtype=np.float32)
    for p, (m, n) in enumerate(pairs):
        w0sym[p] = W0r[m, n] + W0r[n, m]
    w0h = 0.5 * w0sym.reshape(KT, 128, C0).transpose(1, 0, 2).copy()

    w0x = np.zeros((128, C0), dtype=np.float32)
    for m in range(M0):
        w0x[m] = W0r[m, m]
    for p, (m, n) in enumerate(pairs):
        w0x[m] -= 0.5 * w0sym[p]
        w0x[n] -= 0.5 * w0sym[p]

    # w1sb[n, m, o] = W1[m*128+n, o]
    w1sb = W1.reshape(M0, C1, C0).transpose(1, 0, 2).copy()

    return dict(
        selsum=np.ascontiguousarray(selsum.astype(BF16)),
        w0h=np.ascontiguousarray(w0h.astype(BF16)),
        w0x=np.ascontiguousarray(w0x.astype(BF16)),
        w1sb=np.ascontiguousarray(w1sb.astype(BF16)),
        ident=np.ascontiguousarray(np.eye(128, dtype=np.float32).astype(BF16)),
    )


def host_core_inputs(x_c):
    """Per-core prepped tensors from the [BL, M0, D] input shard."""
    # xdT[m, ch*128 + b8*16 + d] = x[ch*8+b8, m, d]  (zero-padded to 128
    # partitions so every K-dim is a full 128)
    xdT = np.zeros((128, R), dtype=np.float32)
    xdT[:M0] = x_c.transpose(1, 0, 2).reshape(M0, R)
    xdT = np.ascontiguousarray(xdT.astype(BF16))
    # 8-way block-diagonal Gram operand (single K=128 matmul per chunk):
    # xc[b8*16+d, ch, b8*40+m] = x[ch*8+b8, m, d]; col m=39 holds 1.0
    xt = x_c.reshape(NCHUNK, BPC, M0, D).transpose(1, 3, 0, 2)  # [b8, d, ch, m]
    xc = np.zeros((BPC, D, NCHUNK, BPC, HW_), dtype=np.float32)
    for b8 in range(BPC):
        xc[b8, :, :, b8, :M0] = xt[b8]
        xc[b8, :, :, b8, M0] = 1.0
    xc = xc.reshape(128, NCHUNK, BPC * HW_)
    return dict(
        xdT=xdT,
        xc=np.ascontiguousarray(xc.astype(BF16)),
    )


def split_sync_waits(nc):
    """Rewrite every instruction carrying >1 sync wait: keep the first wait,
    hoist the rest onto same-engine NoOps inserted immediately before it.
    (walrus in this toolchain rejects >1 sync-wait per instruction.)"""
    import concourse.mybir as mybir

    counter = [0]
    for f in nc.m.functions:
        for bb in f.blocks:
            new_list = []
            changed = False
            for inst in bb.instructions:
                si = inst.sync_info
                waits = list(si.on_wait) if si is not None else []
                if len(waits) > 1:
                    changed = True
                    for w in waits[:-1]:
                        counter[0] += 1
                        nop = mybir.InstNoOp(
                            name=f"WSPLIT-{counter[0]}", ins=[], outs=[]
                        )
                        nop.engine = inst.engine
                        nop.sync_info = mybir.SyncInfo(on_wait=[w], on_update=[])
                        new_list.append(nop)
                    si.on_wait = waits[-1:]
                new_list.append(inst)
            if changed:
                bb.instructions = new_list
    return counter[0]


def build_program(reps=1, split_waits=True, loop_reps=None, cfg=None):
    """loop_reps: if set, wrap a 2-unrolled body in a tc.For_i hardware loop
    (loop_reps must be even; total reps executed = loop_reps) for slope-based
    HW timing with cross-iteration DMA/compute overlap."""
    import contextlib

    cfg = cfg or {}

    import concourse.bass as bass
    import concourse.mybir as mybir
    import concourse.tile as tile

    f32 = mybir.dt.float32
    bf16 = mybir.dt.bfloat16

    if loop_reps is not None:
        unroll = cfg.get("unroll", 2)
        assert loop_reps % unroll == 0
        body_reps, trips = unroll, loop_reps // unroll
    else:
        body_reps, trips = reps, None

    nc = bass.Bass("TRN2", target_bir_lowering=False, debug=False)
    d_xdT = nc.dram_tensor("xdT", [128, R], bf16, kind="ExternalInput")
    d_xc = nc.dram_tensor("xc", [128, NCHUNK, BPC * HW_], bf16, kind="ExternalInput")
    d_sel = nc.dram_tensor("selsum", [128, KT, 128], bf16, kind="ExternalInput")
    d_w0 = nc.dram_tensor("w0h", [128, KT, C0], bf16, kind="ExternalInput")
    d_w0x = nc.dram_tensor("w0x", [128, C0], bf16, kind="ExternalInput")
    d_w1 = nc.dram_tensor("w1sb", [128, M0, C0], bf16, kind="ExternalInput")
    d_id = nc.dram_tensor("ident", [128, 128], bf16, kind="ExternalInput")
    d_out = nc.dram_tensor("out", [BL, C0 + C1], f32, kind="ExternalOutput")

    with tile.TileContext(nc) as tc:
        with (
            tc.tile_pool(name="const", bufs=2) as cpool,
            tc.tile_pool(name="xsqp", bufs=2) as xsqpool,
            tc.tile_pool(name="sqp", bufs=2) as sqpool,
            tc.tile_pool(name="x1p", bufs=2) as x1pool,
            tc.tile_pool(name="hp", bufs=2) as hpool,
            tc.tile_pool(name="outp", bufs=2) as opool,
            tc.tile_pool(name="psA", bufs=2, space="PSUM") as psA,
            tc.tile_pool(name="psB", bufs=2, space="PSUM") as psB,
            tc.tile_pool(name="psC", bufs=2, space="PSUM") as psC,
        ):
            loop_cm = (
                tc.For_i(
                    0,
                    trips,
                    1,
                    hint_engines=(
                        mybir.EngineType.PE,
                        mybir.EngineType.Activation,
                        mybir.EngineType.DVE,
                        mybir.EngineType.SP,
                        mybir.EngineType.Pool,
                    ),
                )
                if trips is not None
                else contextlib.nullcontext()
            )
            with loop_cm:
                for _rep in range(body_reps):
                    xdT = cpool.tile([128, R], bf16, tag="xdT")
                    sel = cpool.tile([128, KT, 128], bf16, tag="sel")
                    w0 = cpool.tile([128, KT, C0], bf16, tag="w0")
                    w0x = cpool.tile([128, C0], bf16, tag="w0x")
                    w1 = cpool.tile([128, M0, C0], bf16, tag="w1")
                    xc = cpool.tile([128, NCHUNK, BPC * HW_], bf16, tag="xc")
                    ident = cpool.tile([128, 128], bf16, tag="ident")
                    # first compute's operands first
                    nc.sync.dma_start(xdT[:, : R // 2], d_xdT[:, : R // 2])
                    nc.sync.dma_start(sel[:], d_sel[:, :, :])
                    nc.sync.dma_start(w0[:], d_w0[:, :, :])
                    nc.sync.dma_start(xdT[:, R // 2 :], d_xdT[:, R // 2 :])
                    nc.sync.dma_start(
                        xc[:, : NCHUNK // 2, :], d_xc[:, : NCHUNK // 2, :]
                    )
                    nc.sync.dma_start(
                        xc[:, NCHUNK // 2 :, :], d_xc[:, NCHUNK // 2 :, :]
                    )
                    nc.sync.dma_start(w0x[:], d_w0x[:, :])
                    nc.sync.dma_start(w1[:, : M0 // 2, :], d_w1[:, : M0 // 2, :])
                    nc.sync.dma_start(w1[:, M0 // 2 :, :], d_w1[:, M0 // 2 :, :])
                    nc.sync.dma_start(ident[:], d_id[:, :])

                    xsq = xsqpool.tile([128, R], bf16, tag="xsq")
                    x1sb = x1pool.tile([128, NCHUNK, C0], bf16, tag="x1sb")
                    hsb = hpool.tile([128, NCHUNK, BPC * HW_], bf16, tag="hsb")
                    hsb3 = hsb[:].rearrange("n c (b m) -> n (c b) m", m=HW_)
                    outsb = opool.tile([128, 2, C0 + C1], f32, tag="outsb")

                    XSQ_ENGINE = (
                        nc.gpsimd.tensor_mul
                        if cfg.get("xsq_pool", False)
                        else nc.vector.tensor_mul
                    )
                    OUT_DMA_QUEUE = (
                        nc.gpsimd.dma_start
                        if cfg.get("out_pool", False)
                        else nc.sync.dma_start
                    )
                    sqs = {}

                    def emit_sel_dual(rc, g):
                        rsl = slice(rc * RC, (rc + 1) * RC)
                        ps = psA.tile([128, 2, RC], f32, tag="sel2")
                        for j in range(2):
                            nc.tensor.matmul(
                                ps[:, j, :],
                                sel[:, 2 * g + j, :],
                                xdT[:, rsl],
                                start=True,
                                stop=True,
                            )
                        sq = sqpool.tile([128, 2, RC], bf16, tag=f"sq{g}")
                        nc.scalar.square(sq[:], ps[:])
                        sqs[(rc, g)] = sq

                    def emit_xsq(rc):
                        rsl = slice(rc * RC, (rc + 1) * RC)
                        # xsq = x*x (SBUF->SBUF)
                        XSQ_ENGINE(
                            xsq[:, rsl], xdT[:, rsl], xdT[:, rsl]
                        )

                    def emit_x1_chunk(x1ps, rc, rs):
                        ch = rc * (RC // 128) + rs
                        csl = slice(rs * 128, (rs + 1) * 128)
                        for t in range(KT):
                            nc.tensor.matmul(
                                x1ps[:, rs, :],
                                sqs[(rc, t // 2)][:, t % 2, csl],
                                w0[:, t, :],
                                start=(t == 0),
                                stop=False,
                            )
                        nc.tensor.matmul(
                            x1ps[:, rs, :],
                            xsq[:, ch * 128 : (ch + 1) * 128],
                            w0x[:],
                            start=False,
                            stop=True,
                        )

                    def emit_gram_chunk(ch):
                        hps = psC.tile([128, BPC * HW_], f32, tag="h")
                        nc.tensor.matmul(
                            hps[:],
                            x1sb[:, ch, :],
                            xc[:, ch, :],
                            start=True,
                            stop=True,
                        )
                        nc.vector.tensor_copy(hsb[:, ch, :], hps[:])

                    def emit_out(bt):
                        btsl = slice(bt * 128, (bt + 1) * 128)
                        o2 = psB.tile([128, C1], f32, tag="x1")
                        for m in range(M0):
                            nc.tensor.matmul(
                                o2[:],
                                hsb3[:, btsl, m],
                                w1[:, m, :],
                                start=(m == 0),
                                stop=(m == M0 - 1),
                            )
                        nc.vector.tensor_copy(outsb[:, bt, C0:], o2[:])
                        o1 = psA.tile([128, C0], bf16, tag="sel2")
                        nc.tensor.transpose(o1[:], hsb3[:, btsl, M0], ident[:])
                        nc.vector.tensor_copy(outsb[:, bt, :C0], o1[:])
                        # store on a side queue so next-iteration input
                        # loads on the sync queue aren't blocked behind it
                        OUT_DMA_QUEUE(d_out[btsl, :], outsb[:, bt, :])

                    # software-pipelined schedule, period rc (RC=512 rows);
                    # gram chunks of rc-1 and x1 chunks of rc are spread
                    # through the period so no engine gates the PE queue head:
                    #   d0 g0 d1 g1 d2 g2 x1c0 g3 x1c1 x1c2 x1c3
                    nch = RC // 128
                    for g in range(3):
                        emit_sel_dual(0, g)
                    emit_xsq(0)
                    for rc in range(NRC):
                        if rc + 1 < NRC:
                            emit_xsq(rc + 1)
                        x1ps = psB.tile([128, nch, C0], f32, tag="x1")
                        gram_base = (rc - 1) * nch

                        def gram(i):
                            if rc - 1 >= 0:
                                emit_gram_chunk(gram_base + i)

                        for g in range(3):
                            if rc + 1 < NRC:
                                emit_sel_dual(rc + 1, g)
                            gram(g)
                        emit_x1_chunk(x1ps, rc, 0)
                        gram(3)
                        for rs in range(1, nch):
                            emit_x1_chunk(x1ps, rc, rs)
                        nc.vector.tensor_copy(
                            x1sb[:, rc * nch : (rc + 1) * nch, :], x1ps[:]
                        )
                        if rc == NRC // 2:
                            emit_out(0)
                    for i in range(nch):
                        emit_gram_chunk((NRC - 1) * nch + i)
                    emit_out(1)

    if split_waits:
        split_sync_waits(nc)
    return nc


def make_in_maps(inputs, W0, W1):
    consts = host_constants(np.asarray(W0), np.asarray(W1))
    in_maps = []
    for c in range(NCORES):
        x_c = np.ascontiguousarray(np.asarray(inputs)[c * BL : (c + 1) * BL])
        m = dict(consts)
        m.update(host_core_inputs(x_c))
        in_maps.append(m)
    return in_maps


_KERNEL_CACHE = {}


def kernel(inputs, W0, W1):
    inputs = np.ascontiguousarray(np.asarray(inputs, dtype=np.float32))
    W0 = np.ascontiguousarray(np.asarray(W0, dtype=np.float32))
    W1 = np.ascontiguousarray(np.asarray(W1, dtype=np.float32))
    assert inputs.shape == (B, M0, D) and W0.shape == (M0 * M0, C0)
    assert W1.shape == (M0 * C0, C1)

    if "nc" not in _KERNEL_CACHE:
        _KERNEL_CACHE["nc"] = build_program()
    nc = _KERNEL_CACHE["nc"]

    in_maps = make_in_maps(inputs, W0, W1)

    from concourse.bass_utils import run_bass_kernel_spmd

    res = run_bass_kernel_spmd(nc, in_maps, core_ids=list(range(NCORES)))
    out = np.concatenate([res.results[c]["out"] for c in range(NCORES)], axis=0)
    return np.ascontiguousarray(out.astype(np.float32))


# revision 21
# speedup vs baseline: 1.2581x; 1.0480x over previous
"""Trainium2 Bass kernel for the CIN (xDeepFM) block.

inputs [2048,39,16] f32, W0 [1521,128], W1 [4992,128] -> out [2048,256] f32.
Data-parallel over the batch axis across 8 NeuronCores; weights replicated.

Per-core pipeline (R = 256 batches x 16 dims = 4096 rows r=(b,d)):
  X1[r,o] = sum_{m<n} w0sym[(m,n),o] x_m x_n + sum_m w0x[m,o] x_m^2 via the
  polarization identity: PE builds pair-sums s=(x_m+x_n) for 741 off-diag
  pairs (6 K-tiles of 128), ACT squares them during PSUM evacuation, PE
  contracts with 0.5*w0sym; the diagonal + correction terms use xsq = x*x
  (computed SBUF->SBUF on the Pool engine) as a 7th K=39 accumulation tile.
  Layer 1 contracts d first: per-batch Gram H_b = X1_b^T [x_b | 1] via
  K=16 matmuls on partition-offset slices (compact x tile, no zero pad);
  out2 accumulates H^T-slices against W1; out1 comes from the ones column
  via a PE transpose. All matmul operands bf16, accumulation fp32.

PE program order is software-pipelined (sel(rc+1); gram(rc-1); x1(rc)) so
the ACT square latency never stalls the PE, and the For_i timing loop wraps
a 2-unrolled body so each iteration's DMAs overlap the other's compute.
"""


import ml_dtypes
import numpy as np

BF16 = ml_dtypes.bfloat16

B, M0, D = 2048, 39, 16
C0, C1 = 128, 128
NCORES = 8
BL = B // NCORES          # 256 batches per core
R = BL * D                # 4096 rows per core
NPAIR = (M0 * (M0 - 1)) // 2   # 741 off-diagonal pairs
KT = (NPAIR + 127) // 128      # 6 K-tiles
RC = 512                       # r-chunk for the sel/square stage
NRC = R // RC                  # 8
NCHUNK = R // 128              # 32 chunks of (8 b x 16 d)
BPC = 128 // D                 # 8 batches per 128-row chunk
HW_ = M0 + 1                   # x columns + ones column (d-sum -> out1)


def host_constants(W0, W1):
    """Core-independent prepped tensors.

    Polarization: x_m*x_n = 0.5*(x_m+x_n)^2 - 0.5*(x_m^2+x_n^2), so
      X1[r,:] = sum_p (0.5*w0sym[p])*sq_p[r] + sum_m w0x[m]*x_m[r]^2
    with sq_p built on the PE via the summed selection matrix and squared
    during the ACT PSUM->SBUF evacuation, and w0x carrying both the
    diagonal W0 terms and the -0.5*(x_m^2+x_n^2) corrections.
    """
    pairs = [(m, n) for m in range(M0) for n in range(m + 1, M0)]
    assert len(pairs) == NPAIR

    selsum = np.zeros((128, KT, 128), dtype=np.float32)
    for p, (m, n) in enumerate(pairs):
        t, q = divmod(p, 128)
        selsum[m, t, q] += 1.0
        selsum[n, t, q] += 1.0

    W0r = W0.reshape(M0, M0, C0)
    w0sym = np.zeros((KT * 128, C0), dtype=np.float32)
    for p, (m, n) in enumerate(pairs):
        w0sym[p] = W0r[m, n] + W0r[n, m]
    w0h = 0.5 * w0sym.reshape(KT, 128, C0).transpose(1, 0, 2).copy()

    w0x = np.zeros((128, C0), dtype=np.float32)
    for m in range(M0):
        w0x[m] = W0r[m, m]
    for p, (m, n) in enumerate(pairs):
        w0x[m] -= 0.5 * w0sym[p]
        w0x[n] -= 0.5 * w0sym[p]

    # w1sb[n, m, o] = W1[m*128+n, o]
    w1sb = W1.reshape(M0, C1, C0).transpose(1, 0, 2).copy()

    return dict(
        selsum=np.ascontiguousarray(selsum.astype(BF16)),
        w0h=np.ascontiguousarray(w0h.astype(BF16)),
        w0x=np.ascontiguousarray(w0x.astype(BF16)),
        w1sb=np.ascontiguousarray(w1sb.astype(BF16)),
        ident=np.ascontiguousarray(np.eye(128, dtype=np.float32).astype(BF16)),
    )


def host_core_inputs(x_c):
    """Per-core prepped tensors from the [BL, M0, D] input shard."""
    # xdT[m, ch*128 + b8*16 + d] = x[ch*8+b8, m, d]  (zero-padded to 128
    # partitions so every K-dim is a full 128)
    xdT = np.zeros((128, R), dtype=np.float32)
    xdT[:M0] = x_c.transpose(1, 0, 2).reshape(M0, R)
    xdT = np.ascontiguousarray(xdT.astype(BF16))
    # compact Gram operand, expanded on-chip into the 8-way block-diagonal
    # tile (zeroed by a Pool memset, filled by 8 strided DMAs):
    # xcc[b8*16+d, ch, m] = x[ch*8+b8, m, d]; col m=39 holds 1.0
    xcc = np.empty((BPC, D, NCHUNK, HW_), dtype=np.float32)
    xcc[:, :, :, :M0] = x_c.reshape(NCHUNK, BPC, M0, D).transpose(1, 3, 0, 2)
    xcc[:, :, :, M0] = 1.0
    return dict(
        xdT=xdT,
        xcc=np.ascontiguousarray(xcc.reshape(128, NCHUNK, HW_).astype(BF16)),
    )


def split_sync_waits(nc):
    """Rewrite every instruction carrying >1 sync wait: keep the first wait,
    hoist the rest onto same-engine NoOps inserted immediately before it.
    (walrus in this toolchain rejects >1 sync-wait per instruction.)"""
    import concourse.mybir as mybir

    counter = [0]
    for f in nc.m.functions:
        for bb in f.blocks:
            new_list = []
            changed = False
            for inst in bb.instructions:
                si = inst.sync_info
                waits = list(si.on_wait) if si is not None else []
                if len(waits) > 1:
                    changed = True
                    for w in waits[:-1]:
                        counter[0] += 1
                        nop = mybir.InstNoOp(
                            name=f"WSPLIT-{counter[0]}", ins=[], outs=[]
                        )
                        nop.engine = inst.engine
                        nop.sync_info = mybir.SyncInfo(on_wait=[w], on_update=[])
                        new_list.append(nop)
                    si.on_wait = waits[-1:]
                new_list.append(inst)
            if changed:
                bb.instructions = new_list
    return counter[0]


def build_program(reps=1, split_waits=True, loop_reps=None, cfg=None):
    """loop_reps: if set, wrap a 2-unrolled body in a tc.For_i hardware loop
    (loop_reps must be even; total reps executed = loop_reps) for slope-based
    HW timing with cross-iteration DMA/compute overlap."""
    import contextlib

    cfg = cfg or {}

    import concourse.bass as bass
    import concourse.mybir as mybir
    import concourse.tile as tile

    f32 = mybir.dt.float32
    bf16 = mybir.dt.bfloat16

    if loop_reps is not None:
        unroll = cfg.get("unroll", 2)
        assert loop_reps % unroll == 0
        body_reps, trips = unroll, loop_reps // unroll
    else:
        body_reps, trips = reps, None

    nc = bass.Bass("TRN2", target_bir_lowering=False, debug=False)
    d_xdT = nc.dram_tensor("xdT", [128, R], bf16, kind="ExternalInput")
    d_xcc = nc.dram_tensor("xcc", [128, NCHUNK, HW_], bf16, kind="ExternalInput")
    d_sel = nc.dram_tensor("selsum", [128, KT, 128], bf16, kind="ExternalInput")
    d_w0 = nc.dram_tensor("w0h", [128, KT, C0], bf16, kind="ExternalInput")
    d_w0x = nc.dram_tensor("w0x", [128, C0], bf16, kind="ExternalInput")
    d_w1 = nc.dram_tensor("w1sb", [128, M0, C0], bf16, kind="ExternalInput")
    d_id = nc.dram_tensor("ident", [128, 128], bf16, kind="ExternalInput")
    d_out = nc.dram_tensor("out", [BL, C0 + C1], f32, kind="ExternalOutput")

    with tile.TileContext(nc) as tc:
        with (
            tc.tile_pool(name="const", bufs=2) as cpool,
            tc.tile_pool(name="xsqp", bufs=2) as xsqpool,
            tc.tile_pool(name="sqp", bufs=2) as sqpool,
            tc.tile_pool(name="x1p", bufs=2) as x1pool,
            tc.tile_pool(name="hp", bufs=2) as hpool,
            tc.tile_pool(name="outp", bufs=2) as opool,
            tc.tile_pool(name="psA", bufs=2, space="PSUM") as psA,
            tc.tile_pool(name="psB", bufs=2, space="PSUM") as psB,
            tc.tile_pool(name="psC", bufs=2, space="PSUM") as psC,
        ):
            loop_cm = (
                tc.For_i(
                    0,
                    trips,
                    1,
                    hint_engines=(
                        mybir.EngineType.PE,
                        mybir.EngineType.Activation,
                        mybir.EngineType.DVE,
                        mybir.EngineType.SP,
                        mybir.EngineType.Pool,
                    ),
                )
                if trips is not None
                else contextlib.nullcontext()
            )
            with loop_cm:
                for _rep in range(body_reps):
                    xdT = cpool.tile([128, R], bf16, tag="xdT")
                    sel = cpool.tile([128, KT, 128], bf16, tag="sel")
                    w0 = cpool.tile([128, KT, C0], bf16, tag="w0")
                    w0x = cpool.tile([128, C0], bf16, tag="w0x")
                    w1 = cpool.tile([128, M0, C0], bf16, tag="w1")
                    xc = cpool.tile([128, NCHUNK, BPC * HW_], bf16, tag="xc")
                    ident = cpool.tile([128, 128], bf16, tag="ident")
                    # first compute's operands first
                    nc.sync.dma_start(xdT[:, : R // 2], d_xdT[:, : R // 2])
                    nc.sync.dma_start(sel[:], d_sel[:, :, :])
                    nc.sync.dma_start(w0[:], d_w0[:, :, :])
                    nc.sync.dma_start(xdT[:, R // 2 :], d_xdT[:, R // 2 :])
                    nc.gpsimd.memset(xc[:], 0.0)
                    for b8 in range(BPC):
                        nc.sync.dma_start(
                            xc[b8 * D : (b8 + 1) * D, :, b8 * HW_ : (b8 + 1) * HW_],
                            d_xcc[b8 * D : (b8 + 1) * D, :, :],
                        )
                    nc.sync.dma_start(w0x[:], d_w0x[:, :])
                    nc.sync.dma_start(w1[:, : M0 // 2, :], d_w1[:, : M0 // 2, :])
                    nc.sync.dma_start(w1[:, M0 // 2 :, :], d_w1[:, M0 // 2 :, :])
                    nc.sync.dma_start(ident[:], d_id[:, :])

                    xsq = xsqpool.tile([128, R], bf16, tag="xsq")
                    x1sb = x1pool.tile([128, NCHUNK, C0], bf16, tag="x1sb")
                    hsb = hpool.tile([128, NCHUNK, BPC * HW_], bf16, tag="hsb")
                    hsb3 = hsb[:].rearrange("n c (b m) -> n (c b) m", m=HW_)
                    outsb = opool.tile([128, 2, C0 + C1], f32, tag="outsb")

                    XSQ_ENGINE = (
                        nc.gpsimd.tensor_mul
                        if cfg.get("xsq_pool", False)
                        else nc.vector.tensor_mul
                    )
                    OUT_DMA_QUEUE = (
                        nc.gpsimd.dma_start
                        if cfg.get("out_pool", False)
                        else nc.sync.dma_start
                    )
                    sqs = {}

                    def emit_sel_dual(rc, g):
                        rsl = slice(rc * RC, (rc + 1) * RC)
                        ps = psA.tile([128, 2, RC], f32, tag="sel2")
                        for j in range(2):
                            nc.tensor.matmul(
                                ps[:, j, :],
                                sel[:, 2 * g + j, :],
                                xdT[:, rsl],
                                start=True,
                                stop=True,
                            )
                        sq = sqpool.tile([128, 2, RC], bf16, tag=f"sq{g}")
                        nc.scalar.square(sq[:], ps[:])
                        sqs[(rc, g)] = sq

                    def emit_xsq(rc):
                        rsl = slice(rc * RC, (rc + 1) * RC)
                        # xsq = x*x (SBUF->SBUF)
                        XSQ_ENGINE(
                            xsq[:, rsl], xdT[:, rsl], xdT[:, rsl]
                        )

                    def emit_x1_chunk(x1ps, rc, rs):
                        ch = rc * (RC // 128) + rs
                        csl = slice(rs * 128, (rs + 1) * 128)
                        for t in range(KT):
                            nc.tensor.matmul(
                                x1ps[:, rs, :],
                                sqs[(rc, t // 2)][:, t % 2, csl],
                                w0[:, t, :],
                                start=(t == 0),
                                stop=False,
                            )
                        nc.tensor.matmul(
                            x1ps[:, rs, :],
                            xsq[:, ch * 128 : (ch + 1) * 128],
                            w0x[:],
                            start=False,
                            stop=True,
                        )

                    def emit_gram_chunk(ch):
                        hps = psC.tile([128, BPC * HW_], f32, tag="h")
                        nc.tensor.matmul(
                            hps[:],
                            x1sb[:, ch, :],
                            xc[:, ch, :],
                            start=True,
                            stop=True,
                        )
                        nc.vector.tensor_copy(hsb[:, ch, :], hps[:])

                    def emit_out(bt):
                        btsl = slice(bt * 128, (bt + 1) * 128)
                        o2 = psB.tile([128, C1], f32, tag="x1")
                        for m in range(M0):
                            nc.tensor.matmul(
                                o2[:],
                                hsb3[:, btsl, m],
                                w1[:, m, :],
                                start=(m == 0),
                                stop=(m == M0 - 1),
                            )
                        nc.vector.tensor_copy(outsb[:, bt, C0:], o2[:])
                        o1 = psA.tile([128, C0], bf16, tag="sel2")
                        nc.tensor.transpose(o1[:], hsb3[:, btsl, M0], ident[:])
                        nc.vector.tensor_copy(outsb[:, bt, :C0], o1[:])
                        # store on a side queue so next-iteration input
                        # loads on the sync queue aren't blocked behind it
                        OUT_DMA_QUEUE(d_out[btsl, :], outsb[:, bt, :])

                    # software-pipelined schedule, period rc (RC=512 rows);
                    # gram chunks of rc-1 and x1 chunks of rc are spread
                    # through the period so no engine gates the PE queue head:
                    #   d0 g0 d1 g1 d2 g2 x1c0 g3 x1c1 x1c2 x1c3
                    nch = RC // 128
                    for g in range(3):
                        emit_sel_dual(0, g)
                    emit_xsq(0)
                    for rc in range(NRC):
                        if rc + 1 < NRC:
                            emit_xsq(rc + 1)
                        x1ps = psB.tile([128, nch, C0], f32, tag="x1")
                        gram_base = (rc - 1) * nch

                        def gram(i):
                            if rc - 1 >= 0:
                                emit_gram_chunk(gram_base + i)

                        for g in range(3):
                            if rc + 1 < NRC:
                                emit_sel_dual(rc + 1, g)
                            gram(g)
                        emit_x1_chunk(x1ps, rc, 0)
                        gram(3)
                        for rs in range(1, nch):
                            emit_x1_chunk(x1ps, rc, rs)
                        nc.vector.tensor_copy(
                            x1sb[:, rc * nch : (rc + 1) * nch, :], x1ps[:]
                        )
                        if rc == NRC // 2:
                            emit_out(0)
                    for i in range(nch):
                        emit_gram_chunk((NRC - 1) * nch + i)
                    emit_out(1)

    if split_waits:
        split_sync_waits(nc)
    return nc


def make_in_maps(inputs, W0, W1):
    consts = host_constants(np.asarray(W0), np.asarray(W1))
    in_maps = []
    for c in range(NCORES):
        x_c = np.ascontiguousarray(np.asarray(inputs)[c * BL : (c + 1) * BL])
        m = dict(consts)
        m.update(host_core_inputs(x_c))
        in_maps.append(m)
    return in_maps


_KERNEL_CACHE = {}


def kernel(inputs, W0, W1):
    inputs = np.ascontiguousarray(np.asarray(inputs, dtype=np.float32))
    W0 = np.ascontiguousarray(np.asarray(W0, dtype=np.float32))
    W1 = np.ascontiguousarray(np.asarray(W1, dtype=np.float32))
    assert inputs.shape == (B, M0, D) and W0.shape == (M0 * M0, C0)
    assert W1.shape == (M0 * C0, C1)

    if "nc" not in _KERNEL_CACHE:
        _KERNEL_CACHE["nc"] = build_program()
    nc = _KERNEL_CACHE["nc"]

    in_maps = make_in_maps(inputs, W0, W1)

    from concourse.bass_utils import run_bass_kernel_spmd

    res = run_bass_kernel_spmd(nc, in_maps, core_ids=list(range(NCORES)))
    out = np.concatenate([res.results[c]["out"] for c in range(NCORES)], axis=0)
    return np.ascontiguousarray(out.astype(np.float32))
